# revision 1
# baseline (speedup 1.0000x reference)
"""NT-Xent / SimCLR contrastive loss on 8 Trainium2 NeuronCores.

Math (reference):
  z = concat(proj_1, proj_2)            # [2N, D], 2N=8192, D=128
  zn = z / ||z||                        # row L2-normalize
  sim = zn @ zn.T                       # [2N, 2N]
  denom_i   = sum_{j != i} exp(sim_ij / T)
  pos_i     = sim[i, (i+N) mod 2N]
  loss      = mean_i( log(denom_i) - pos_i / T )

Device decomposition (row-parallel over 8 cores, core c owns rows
[c*1024, (c+1)*1024)):
  - every core gets the full z in bf16, natural layout packed
    [128, 64, 128] with z[128*t + p, d] at [p, t, d]; it computes
    1/||row|| via square->reduce->exp(-0.5*ln(.)) entirely off the
    ScalarE hot path, scales rows, and DMA-transposes tiles into
    zn^T [128(d), 8192(sample)] for the TensorE.
  - gram block rows: sim_block = (own 1024 rows) @ zn^T via 128 matmuls
    (K=128, N=512, fp32 PSUM).
  - exp + row-sum fused on ScalarE: activation(Exp, scale=1/T,
    accum_out=per-partition sum); the exp values themselves are
    discarded (only row sums are needed).
  - denom_i = rowsum - e^2 (self-sim term; sim_ii == 1 to well within
    tolerance), then log(denom) on ScalarE.
  - sum_i pos_i = 2 * <Zn1, Zn2> elementwise (no diagonal extraction
    needed); each core computes the full dot, host averages.
Host: loss = (sum log(denom) - (2/T) * <Zn1,Zn2>) / 8192.
"""

import numpy as np

P = 128          # partitions / feature dim
NS = 8192        # total samples (2N)
D = 128          # feature dim
NCORES = 8
RB = NS // NCORES    # 1024 rows per core
MT = RB // P         # 8 m-tiles per core
NT = NS // P         # 64 sample tiles
TEMP = 0.5
INV_T = 1.0 / TEMP   # 2.0
GT = 8               # sample tiles per pipeline group
NGROUPS = NT // GT   # 8
NFREE = 512          # matmul moving free dim (fp32 PSUM: one bank)
ACT_CHUNK = 2048     # ScalarE exp chunk (4 PSUM banks)

_CACHE = {}


def _ensure_paths():
    import sys
    for p in ("/root/.axon_site", "/root/.axon_site/_ro/trn_rl_repo",
              "/root/.axon_site/_ro/pypackages", "/opt/trn_rl_repo", "/opt/pypackages"):
        if p not in sys.path:
            sys.path.append(p)


def _build():
    _ensure_paths()
    import concourse.bass as bass
    import concourse.bacc as bacc
    import concourse.mybir as mybir
    import concourse.tile as tile

    dt_bf = mybir.dt.bfloat16
    dt_f32 = mybir.dt.float32
    AFT = mybir.ActivationFunctionType
    AX = mybir.AxisListType

    nc = bacc.Bacc("TRN2", target_bir_lowering=False, debug=False,
                   num_devices=NCORES)

    znat_d = nc.dram_tensor("znat", [P, NT, P], dt_bf, kind="ExternalInput")
    mynat_d = nc.dram_tensor("mynat", [P, MT, P], dt_bf, kind="ExternalInput")
    ld_d = nc.dram_tensor("out_ld", [P, MT], dt_f32, kind="ExternalOutput")
    pos_d = nc.dram_tensor("out_pos", [P, 1], dt_f32, kind="ExternalOutput")

    with tile.TileContext(nc) as tc:
        with (
            tc.tile_pool(name="big", bufs=1) as big,
            tc.tile_pool(name="work", bufs=2) as work,
            tc.tile_pool(name="psum", bufs=2, space=bass.MemorySpace.PSUM) as psum,
        ):
            zn_t = big.tile([P, NS], dt_bf, tag="zn_t")    # zn^T [d, sample]
            my_t = big.tile([P, RB], dt_bf, tag="my_t")    # own rows, transposed
            rs = big.tile([P, MT * 4], dt_f32, tag="rs")   # exp row-sum slots

            # ---- own-rows path (small; feeds matmul stationary) ----
            myna = big.tile([P, MT, P], dt_bf, tag="myna")
            nc.gpsimd.dma_start(myna[:], mynat_d[:])
            sqm = big.tile([P, MT, P], dt_bf, tag="sqm")
            nc.vector.tensor_mul(sqm[:], myna[:], myna[:])
            ssm = big.tile([P, MT], dt_f32, tag="ssm")
            nc.vector.reduce_sum(ssm[:], sqm[:], axis=AX.X)
            lnm = big.tile([P, MT], dt_f32, tag="lnm")
            nc.scalar.activation(lnm[:], ssm[:], AFT.Ln)
            ninvm = big.tile([P, MT], dt_f32, tag="ninvm")
            nc.scalar.activation(ninvm[:], lnm[:], AFT.Exp, scale=-0.5)
            mynn = big.tile([P, MT, P], dt_bf, tag="mynn")
            for m in range(MT):
                nc.vector.tensor_scalar_mul(mynn[:, m, :], myna[:, m, :],
                                            ninvm[:, m:m + 1])
            for m in range(MT):
                nc.sync.dma_start_transpose(my_t[:, m * P:(m + 1) * P],
                                            mynn[:, m, :])

            # ---- full-z normalize pipeline (grouped for overlap) ----
            zna = big.tile([P, NT, P], dt_bf, tag="zna")
            ss = big.tile([P, NT], dt_f32, tag="ss")
            lnb = big.tile([P, NT], dt_f32, tag="lnb")
            ninv = big.tile([P, NT], dt_f32, tag="ninv")
            for g in range(NGROUPS):
                sl = slice(g * GT, (g + 1) * GT)
                nc.gpsimd.dma_start(zna[:, sl, :], znat_d[:, sl, :])
                sqg = work.tile([P, GT, P], dt_bf, tag="sq")
                nc.vector.tensor_mul(sqg[:], zna[:, sl, :], zna[:, sl, :])
                nc.vector.reduce_sum(ss[:, sl], sqg[:], axis=AX.X)
                nc.scalar.activation(lnb[:, sl], ss[:, sl], AFT.Ln)
                nc.scalar.activation(ninv[:, sl], lnb[:, sl], AFT.Exp,
                                     scale=-0.5)
                znng = work.tile([P, GT, P], dt_bf, tag="znn")
                for t in range(GT):
                    gt = g * GT + t
                    nc.vector.tensor_scalar_mul(znng[:, t, :], zna[:, gt, :],
                                                ninv[:, gt:gt + 1])
                    nc.sync.dma_start_transpose(zn_t[:, gt * P:(gt + 1) * P],
                                                znng[:, t, :])

            # ---- gram rows + fused exp/row-sum ----
            nchunks = NS // ACT_CHUNK              # 4 chunks of 2048 cols
            nmm = ACT_CHUNK // NFREE               # 4 matmuls per chunk
            for q in range(nchunks):
                for m in range(MT):
                    ps = psum.tile([P, ACT_CHUNK], dt_f32, tag="ps")
                    for j in range(nmm):
                        c0 = q * ACT_CHUNK + j * NFREE
                        nc.tensor.matmul(ps[:, j * NFREE:(j + 1) * NFREE],
                                         my_t[:, m * P:(m + 1) * P],
                                         zn_t[:, c0:c0 + NFREE],
                                         start=True, stop=True)
                    esc = work.tile([P, ACT_CHUNK], dt_bf, tag="esc")
                    idx = m * nchunks + q
                    nc.scalar.activation(esc[:], ps[:], AFT.Exp, scale=INV_T,
                                         accum_out=rs[:, idx:idx + 1])

            # ---- denominators -> log ----
            rsum = big.tile([P, MT], dt_f32, tag="rsum")
            nc.vector.reduce_sum(rsum[:], rs[:].rearrange("p (m q) -> p m q",
                                                          q=nchunks), axis=AX.X)
            den = big.tile([P, MT], dt_f32, tag="den")
            nc.vector.tensor_scalar_add(den[:], rsum[:],
                                        -float(np.exp(2.0)))
            ldb = big.tile([P, MT], dt_f32, tag="ldb")
            nc.scalar.activation(ldb[:], den[:], AFT.Ln)
            nc.gpsimd.dma_start(ld_d[:], ldb[:])

            # ---- positives: <Zn1, Zn2> ----
            pp = big.tile([P, NS // 2], dt_bf, tag="pp")
            nc.vector.tensor_mul(pp[:], zn_t[:, :NS // 2], zn_t[:, NS // 2:])
            posb = big.tile([P, 1], dt_f32, tag="posb")
            nc.vector.reduce_sum(posb[:], pp[:], axis=AX.X)
            nc.gpsimd.dma_start(pos_d[:], posb[:])

    nc.compile()
    return nc


def get_nc():
    if "nc" not in _CACHE:
        _CACHE["nc"] = _build()
    return _CACHE["nc"]


def make_in_maps(proj_1: np.ndarray, proj_2: np.ndarray):
    import ml_dtypes
    z = np.concatenate([np.asarray(proj_1), np.asarray(proj_2)], axis=0)
    zb = z.astype(ml_dtypes.bfloat16)
    # natural packed layout: [p, t, d] = z[128 t + p, d]
    znat = np.ascontiguousarray(zb.reshape(NT, P, P).transpose(1, 0, 2))
    in_maps = []
    for c in range(NCORES):
        myn = np.ascontiguousarray(
            zb[c * RB:(c + 1) * RB].reshape(MT, P, P).transpose(1, 0, 2))
        in_maps.append({"znat": znat, "mynat": myn})
    return in_maps


def finish(results) -> np.ndarray:
    ld_sum = 0.0
    pos_vals = []
    for r in results:
        ld_sum += float(np.asarray(r["out_ld"], dtype=np.float64).sum())
        pos_vals.append(float(np.asarray(r["out_pos"], dtype=np.float64).sum()))
    pos_dot = float(np.mean(pos_vals))
    loss = (ld_sum - 2.0 * INV_T * pos_dot) / float(NS)
    return np.float32(loss)


def kernel(proj_1: np.ndarray, proj_2: np.ndarray) -> np.ndarray:
    _ensure_paths()
    from concourse.bass_utils import run_bass_kernel_spmd
    nc = get_nc()
    in_maps = make_in_maps(proj_1, proj_2)
    res = run_bass_kernel_spmd(nc, in_maps, core_ids=list(range(NCORES)))
    return finish(res.results)


# revision 2
# speedup vs baseline: 1.0475x; 1.0475x over previous
"""NT-Xent / SimCLR contrastive loss on 8 Trainium2 NeuronCores.

Math (reference):
  z = concat(proj_1, proj_2)            # [2N, D], 2N=8192, D=128
  zn = z / ||z||                        # row L2-normalize
  sim = zn @ zn.T                       # [2N, 2N]
  denom_i   = sum_{j != i} exp(sim_ij / T)
  pos_i     = sim[i, (i+N) mod 2N]
  loss      = mean_i( log(denom_i) - pos_i / T )

Device decomposition (row-parallel over 8 cores, core c owns rows
[c*1024, (c+1)*1024)). Each core receives the full z twice: natural
packed [128, 64, 128] (z[128t+p, d] at [p,t,d]) for the row norms, and
raw-transposed z^T [128(d), 8192(sample)] for the GEMM; plus its own
1024 rows natural-packed for the matmul stationary.

  - norms: square -> free-dim reduce -> ACT Sqrt -> DVE reciprocal,
    giving compact ninv [128, 64] (sample s=128t+p at [p, t]).
  - one 128x128 DMA-transpose + a reshape DMA puts ninv in sample order
    [1, 8192]; a DRAM bounce + broadcast-AP DMA replicates it to
    [128, 8192]; one DVE multiply forms zn^T = z^T * ninv_b.
  - own rows: normalize in natural layout (per-partition scalar mul),
    8 DMA-transposes -> my_t [128(d), 1024].
  - gram block rows: 128 matmuls (K=128, N=512, fp32 PSUM), exp +
    row-sum fused on ScalarE via activation(Exp, scale=1/T, accum_out);
    exp values are discarded, only row sums kept.
  - denom_i = rowsum - e^2 (self-sim), log on ScalarE.
  - sum_i pos_i = 2*<Zn1,Zn2> elementwise; host averages cores.
Host: loss = (sum log(denom) - (2/T) * <Zn1,Zn2>) / 8192.
"""

import numpy as np

P = 128          # partitions / feature dim
NS = 8192        # total samples (2N)
D = 128          # feature dim
NCORES = 8
RB = NS // NCORES    # 1024 rows per core
MT = RB // P         # 8 m-tiles per core
NT = NS // P         # 64 sample tiles
TEMP = 0.5
INV_T = 1.0 / TEMP   # 2.0
GT = 8               # sample tiles per pipeline group
NGROUPS = NT // GT   # 8
NFREE = 512          # matmul moving free dim (fp32 PSUM: one bank)
ACT_CHUNK = 2048     # ScalarE exp chunk (4 PSUM banks)

_CACHE = {}


def _ensure_paths():
    import sys
    for p in ("/root/.axon_site", "/root/.axon_site/_ro/trn_rl_repo",
              "/root/.axon_site/_ro/pypackages", "/opt/trn_rl_repo", "/opt/pypackages"):
        if p not in sys.path:
            sys.path.append(p)


def _build():
    _ensure_paths()
    import concourse.bass as bass
    import concourse.bacc as bacc
    import concourse.mybir as mybir
    import concourse.tile as tile

    dt_bf = mybir.dt.bfloat16
    dt_f32 = mybir.dt.float32
    AFT = mybir.ActivationFunctionType
    AX = mybir.AxisListType

    nc = bacc.Bacc("TRN2", target_bir_lowering=False, debug=False,
                   num_devices=NCORES)

    znat_d = nc.dram_tensor("znat", [P, NT, P], dt_bf, kind="ExternalInput")
    zt_d = nc.dram_tensor("zt", [P, NS], dt_bf, kind="ExternalInput")
    mynat_d = nc.dram_tensor("mynat", [P, MT, P], dt_bf, kind="ExternalInput")
    ld_d = nc.dram_tensor("out_ld", [P, MT], dt_f32, kind="ExternalOutput")
    pos_d = nc.dram_tensor("out_pos", [P, 1], dt_f32, kind="ExternalOutput")
    ninv_dram = nc.dram_tensor("ninv_row_scratch", [1, NS], dt_bf)

    with tile.TileContext(nc) as tc:
        with (
            tc.tile_pool(name="big", bufs=1) as big,
            tc.tile_pool(name="work", bufs=2) as work,
            tc.tile_pool(name="psum", bufs=2, space=bass.MemorySpace.PSUM) as psum,
        ):
            zt = big.tile([P, NS], dt_bf, tag="zt")        # z^T raw
            zn_t = big.tile([P, NS], dt_bf, tag="zn_t")    # zn^T
            my_t = big.tile([P, RB], dt_bf, tag="my_t")    # own rows ^T
            rs = big.tile([P, MT * 4], dt_f32, tag="rs")   # exp row-sum slots

            # ---- own-rows path (small; feeds matmul stationary) ----
            myna = big.tile([P, MT, P], dt_bf, tag="myna")
            nc.gpsimd.dma_start(myna[:], mynat_d[:])
            sqm = big.tile([P, MT, P], dt_bf, tag="sqm")
            nc.vector.tensor_mul(sqm[:], myna[:], myna[:])
            ssm = big.tile([P, MT], dt_f32, tag="ssm")
            nc.vector.reduce_sum(ssm[:], sqm[:], axis=AX.X)
            snm = big.tile([P, MT], dt_f32, tag="snm")
            nc.scalar.activation(snm[:], ssm[:], AFT.Sqrt)
            ninvm = big.tile([P, MT], dt_f32, tag="ninvm")
            nc.vector.reciprocal(ninvm[:], snm[:])
            mynn = big.tile([P, MT, P], dt_bf, tag="mynn")
            for m in range(MT):
                nc.vector.tensor_scalar_mul(mynn[:, m, :], myna[:, m, :],
                                            ninvm[:, m:m + 1])
            for m in range(MT):
                nc.sync.dma_start_transpose(my_t[:, m * P:(m + 1) * P],
                                            mynn[:, m, :])

            # ---- full-z norms (grouped), compact ninv [128, 64+pad] ----
            zna = big.tile([P, NT, P], dt_bf, tag="zna")
            ss = big.tile([P, NT], dt_f32, tag="ss")
            sn = big.tile([P, NT], dt_f32, tag="sn")
            ninvf = big.tile([P, NT], dt_f32, tag="ninvf")
            ninvb = big.tile([P, P], dt_bf, tag="ninvb")   # cols 64: pad
            nc.gpsimd.memset(ninvb[:, NT:], 1.0)
            for g in range(NGROUPS):
                sl = slice(g * GT, (g + 1) * GT)
                nc.gpsimd.dma_start(zna[:, sl, :], znat_d[:, sl, :])
                sqg = work.tile([P, GT, P], dt_bf, tag="sq")
                nc.vector.tensor_mul(sqg[:], zna[:, sl, :], zna[:, sl, :])
                nc.vector.reduce_sum(ss[:, sl], sqg[:], axis=AX.X)
                nc.scalar.activation(sn[:, sl], ss[:, sl], AFT.Sqrt)
                nc.vector.reciprocal(ninvf[:, sl], sn[:, sl])
                nc.vector.tensor_copy(ninvb[:, sl], ninvf[:, sl])

            # ---- broadcast ninv to [128, 8192] in sample order ----
            ninv_t = big.tile([P, P], dt_bf, tag="ninv_t")
            nc.sync.dma_start_transpose(ninv_t[:], ninvb[:])
            # sample s = 128 t + p lives at ninv_t[t, p]; rows 0..64 valid
            nc.gpsimd.dma_start(ninv_dram[0:1, :], ninv_t[0:NT, :])
            ninv_b = big.tile([P, NS], dt_bf, tag="ninv_b")
            nq = NS // 4
            for h in range(4):
                src = ninv_dram[0:1, h * nq:(h + 1) * nq].broadcast_to([P, nq])
                nc.gpsimd.dma_start(ninv_b[:, h * nq:(h + 1) * nq], src)

            # ---- z^T load + normalize ----
            for h in range(4):
                nc.gpsimd.dma_start(zt[:, h * nq:(h + 1) * nq],
                                    zt_d[:, h * nq:(h + 1) * nq])
            for h in range(4):
                nc.vector.tensor_mul(zn_t[:, h * nq:(h + 1) * nq],
                                     zt[:, h * nq:(h + 1) * nq],
                                     ninv_b[:, h * nq:(h + 1) * nq])

            # ---- gram rows + fused exp/row-sum ----
            nchunks = NS // ACT_CHUNK              # 4 chunks of 2048 cols
            nmm = ACT_CHUNK // NFREE               # 4 matmuls per chunk
            for q in range(nchunks):
                for m in range(MT):
                    ps = psum.tile([P, ACT_CHUNK], dt_f32, tag="ps")
                    for j in range(nmm):
                        c0 = q * ACT_CHUNK + j * NFREE
                        nc.tensor.matmul(ps[:, j * NFREE:(j + 1) * NFREE],
                                         my_t[:, m * P:(m + 1) * P],
                                         zn_t[:, c0:c0 + NFREE],
                                         start=True, stop=True)
                    esc = work.tile([P, ACT_CHUNK], dt_bf, tag="esc")
                    idx = q * MT + m
                    nc.scalar.activation(esc[:], ps[:], AFT.Exp, scale=INV_T,
                                         accum_out=rs[:, idx:idx + 1])

            # ---- denominators -> log ----
            rsum = big.tile([P, MT], dt_f32, tag="rsum")
            nc.vector.reduce_sum(rsum[:], rs[:].rearrange("p (q m) -> p m q",
                                                          m=MT), axis=AX.X)
            den = big.tile([P, MT], dt_f32, tag="den")
            nc.vector.tensor_scalar_add(den[:], rsum[:],
                                        -float(np.exp(2.0)))
            ldb = big.tile([P, MT], dt_f32, tag="ldb")
            nc.scalar.activation(ldb[:], den[:], AFT.Ln)
            nc.gpsimd.dma_start(ld_d[:], ldb[:])

            # ---- positives: <Zn1, Zn2> ----
            pp = big.tile([P, NS // 2], dt_bf, tag="pp")
            nc.vector.tensor_mul(pp[:], zn_t[:, :NS // 2], zn_t[:, NS // 2:])
            posb = big.tile([P, 1], dt_f32, tag="posb")
            nc.vector.reduce_sum(posb[:], pp[:], axis=AX.X)
            nc.gpsimd.dma_start(pos_d[:], posb[:])

    nc.compile()
    return nc


def get_nc():
    if "nc" not in _CACHE:
        _CACHE["nc"] = _build()
    return _CACHE["nc"]


def make_in_maps(proj_1: np.ndarray, proj_2: np.ndarray):
    import ml_dtypes
    z = np.concatenate([np.asarray(proj_1), np.asarray(proj_2)], axis=0)
    zb = z.astype(ml_dtypes.bfloat16)
    # natural packed layout: [p, t, d] = z[128 t + p, d]
    znat = np.ascontiguousarray(zb.reshape(NT, P, P).transpose(1, 0, 2))
    ztr = np.ascontiguousarray(zb.T)
    in_maps = []
    for c in range(NCORES):
        myn = np.ascontiguousarray(
            zb[c * RB:(c + 1) * RB].reshape(MT, P, P).transpose(1, 0, 2))
        in_maps.append({"znat": znat, "zt": ztr, "mynat": myn})
    return in_maps


def finish(results) -> np.ndarray:
    ld_sum = 0.0
    pos_vals = []
    for r in results:
        ld_sum += float(np.asarray(r["out_ld"], dtype=np.float64).sum())
        pos_vals.append(float(np.asarray(r["out_pos"], dtype=np.float64).sum()))
    pos_dot = float(np.mean(pos_vals))
    loss = (ld_sum - 2.0 * INV_T * pos_dot) / float(NS)
    return np.float32(loss)


def kernel(proj_1: np.ndarray, proj_2: np.ndarray) -> np.ndarray:
    _ensure_paths()
    from concourse.bass_utils import run_bass_kernel_spmd
    nc = get_nc()
    in_maps = make_in_maps(proj_1, proj_2)
    res = run_bass_kernel_spmd(nc, in_maps, core_ids=list(range(NCORES)))
    return finish(res.results)


# revision 3
# speedup vs baseline: 1.1001x; 1.0503x over previous
"""NT-Xent / SimCLR contrastive loss on 8 Trainium2 NeuronCores.

Math (reference):
  z = concat(proj_1, proj_2)            # [2N, D], 2N=8192, D=128
  zn = z / ||z||                        # row L2-normalize
  sim = zn @ zn.T                       # [2N, 2N]
  denom_i   = sum_{j != i} exp(sim_ij / T)
  pos_i     = sim[i, (i+N) mod 2N]
  loss      = mean_i( log(denom_i) - pos_i / T )

Device decomposition (row-parallel over 8 cores, core c owns rows
[c*1024, (c+1)*1024)). Inputs per core: full z natural-packed
[128, 64, 128] (z[128t+p, d] at [p,t,d]) for row norms, full raw z^T
[128(d), 8192(sample)] for the GEMM moving operand, and the core's own
1024 columns of raw z^T for the stationary.

Normalization never materializes zn in natural layout: compact
1/||row|| ([128, nt] via square -> reduce -> ACT Sqrt -> DVE recip) is
re-ordered to sample order with one padded 128x128 DMA-transpose +
reshape DMA, bounced through DRAM, DMA-broadcast to all 128 partitions,
and applied with a single elementwise multiply against raw z^T. Four
independent 2048-column pipelines overlap DMA/DVE/ACT/transpose so the
GEMM starts ~10us in.

Gram phase: 128 matmuls (K=128, N=512, fp32 PSUM); exp + row-sum fused
on ScalarE via activation(Exp, scale=1/T, accum_out=...); exp values
are discarded. denom = rowsum - e^2 (self-sim), log on ScalarE.
positives: sum_i pos_i = 2*<Zn1,Zn2> elementwise, no diagonal
extraction. Host: loss = (sum log(denom) - (2/T)*<Zn1,Zn2>) / 8192.
"""

import numpy as np

P = 128          # partitions / feature dim
NS = 8192        # total samples (2N)
D = 128          # feature dim
NCORES = 8
RB = NS // NCORES    # 1024 rows per core
MT = RB // P         # 8 m-tiles per core
NT = NS // P         # 64 sample tiles
TEMP = 0.5
INV_T = 1.0 / TEMP   # 2.0
NFREE = 512          # matmul moving free dim (fp32 PSUM: one bank)
ACT_CHUNK = 2048     # ScalarE exp chunk (4 PSUM banks)
NCHUNK = NS // ACT_CHUNK   # 4 column chunks
CT = ACT_CHUNK // P        # 16 sample tiles per chunk

_CACHE = {}


def _ensure_paths():
    import sys
    for p in ("/root/.axon_site", "/root/.axon_site/_ro/trn_rl_repo",
              "/root/.axon_site/_ro/pypackages", "/opt/trn_rl_repo", "/opt/pypackages"):
        if p not in sys.path:
            sys.path.append(p)


def _build():
    _ensure_paths()
    import concourse.bass as bass
    import concourse.bacc as bacc
    import concourse.mybir as mybir
    import concourse.tile as tile

    dt_bf = mybir.dt.bfloat16
    dt_f32 = mybir.dt.float32
    AFT = mybir.ActivationFunctionType
    AX = mybir.AxisListType

    nc = bacc.Bacc("TRN2", target_bir_lowering=False, debug=False,
                   num_devices=NCORES)

    znat_d = nc.dram_tensor("znat", [P, NT, P], dt_bf, kind="ExternalInput")
    zt_d = nc.dram_tensor("zt", [P, NS], dt_bf, kind="ExternalInput")
    myt_d = nc.dram_tensor("myt", [P, RB], dt_bf, kind="ExternalInput")
    mynat_d = nc.dram_tensor("mynat", [P, MT, P], dt_bf, kind="ExternalInput")
    ld_d = nc.dram_tensor("out_ld", [P, MT], dt_f32, kind="ExternalOutput")
    pos_d = nc.dram_tensor("out_pos", [P, 1], dt_f32, kind="ExternalOutput")
    ninv_dram = nc.dram_tensor("ninv_row_scratch", [1, NS], dt_bf)
    minv_dram = nc.dram_tensor("minv_row_scratch", [1, RB], dt_bf)

    with tile.TileContext(nc) as tc:
        with (
            tc.tile_pool(name="big", bufs=1) as big,
            tc.tile_pool(name="work", bufs=2) as work,
            tc.tile_pool(name="psum", bufs=2, space=bass.MemorySpace.PSUM) as psum,
        ):
            zt = big.tile([P, NS], dt_bf, tag="zt")        # z^T raw
            zn_t = big.tile([P, NS], dt_bf, tag="zn_t")    # zn^T
            my_t = big.tile([P, RB], dt_bf, tag="my_t")    # own rows ^T
            rs = big.tile([P, MT * NCHUNK], dt_f32, tag="rs")

            # ---- own-rows norms + broadcast-normalize (early) ----
            myt_raw = big.tile([P, RB], dt_bf, tag="myt_raw")
            nc.gpsimd.dma_start(myt_raw[:], myt_d[:])
            myna = big.tile([P, MT, P], dt_bf, tag="myna")
            nc.gpsimd.dma_start(myna[:], mynat_d[:])
            sqm = big.tile([P, MT, P], dt_bf, tag="sqm")
            nc.vector.tensor_mul(sqm[:], myna[:], myna[:])
            ssm = big.tile([P, MT], dt_f32, tag="ssm")
            nc.vector.reduce_sum(ssm[:], sqm[:], axis=AX.X)
            snm = big.tile([P, MT], dt_f32, tag="snm")
            nc.scalar.activation(snm[:], ssm[:], AFT.Sqrt)
            minvb = big.tile([P, P], dt_bf, tag="minvb")   # cols 8: pad
            nc.gpsimd.memset(minvb[:, MT:], 1.0)
            ninvm = big.tile([P, MT], dt_f32, tag="ninvm")
            nc.vector.reciprocal(ninvm[:], snm[:])
            nc.vector.tensor_copy(minvb[:, :MT], ninvm[:])
            minv_t = big.tile([P, P], dt_bf, tag="minv_t")
            nc.sync.dma_start_transpose(minv_t[:], minvb[:])
            nc.gpsimd.dma_start(minv_dram[0:1, :], minv_t[0:MT, :])
            minv_b = big.tile([P, RB], dt_bf, tag="minv_b")
            nc.gpsimd.dma_start(minv_b[:],
                                minv_dram[0:1, :].broadcast_to([P, RB]))
            nc.vector.tensor_mul(my_t[:], myt_raw[:], minv_b[:])

            # ---- full-z norms + normalize, 4 independent chunks ----
            zna = big.tile([P, NT, P], dt_bf, tag="zna")
            ss = big.tile([P, NT], dt_f32, tag="ss")
            sn = big.tile([P, NT], dt_f32, tag="sn")
            ninvf = big.tile([P, NT], dt_f32, tag="ninvf")
            ninv_b = big.tile([P, NS], dt_bf, tag="ninv_b")
            for h in range(NCHUNK):
                tsl = slice(h * CT, (h + 1) * CT)          # tile indices
                csl = slice(h * ACT_CHUNK, (h + 1) * ACT_CHUNK)  # columns
                nc.gpsimd.dma_start(zna[:, tsl, :], znat_d[:, tsl, :])
                nc.gpsimd.dma_start(zt[:, csl], zt_d[:, csl])
                sqg = work.tile([P, CT, P], dt_bf, tag="sq")
                nc.vector.tensor_mul(sqg[:], zna[:, tsl, :], zna[:, tsl, :])
                nc.vector.reduce_sum(ss[:, tsl], sqg[:], axis=AX.X)
                nc.scalar.activation(sn[:, tsl], ss[:, tsl], AFT.Sqrt)
                nc.vector.reciprocal(ninvf[:, tsl], sn[:, tsl])
                nvb = work.tile([P, P], dt_bf, tag="nvb")  # cols CT: pad
                nc.gpsimd.memset(nvb[:, CT:], 1.0)
                nc.vector.tensor_copy(nvb[:, :CT], ninvf[:, tsl])
                nvt = work.tile([P, P], dt_bf, tag="nvt")
                nc.sync.dma_start_transpose(nvt[:], nvb[:])
                nc.gpsimd.dma_start(ninv_dram[0:1, csl], nvt[0:CT, :])
                nc.gpsimd.dma_start(ninv_b[:, csl],
                                    ninv_dram[0:1, csl].broadcast_to(
                                        [P, ACT_CHUNK]))
                nc.vector.tensor_mul(zn_t[:, csl], zt[:, csl],
                                     ninv_b[:, csl])

            # ---- gram rows + fused exp/row-sum ----
            nmm = ACT_CHUNK // NFREE               # 4 matmuls per chunk
            for q in range(NCHUNK):
                for m in range(MT):
                    ps = psum.tile([P, ACT_CHUNK], dt_f32, tag="ps")
                    for j in range(nmm):
                        c0 = q * ACT_CHUNK + j * NFREE
                        nc.tensor.matmul(ps[:, j * NFREE:(j + 1) * NFREE],
                                         my_t[:, m * P:(m + 1) * P],
                                         zn_t[:, c0:c0 + NFREE],
                                         start=True, stop=True)
                    esc = work.tile([P, ACT_CHUNK], dt_bf, tag="esc")
                    idx = q * MT + m
                    nc.scalar.activation(esc[:], ps[:], AFT.Exp, scale=INV_T,
                                         accum_out=rs[:, idx:idx + 1])

            # ---- denominators -> log ----
            rsum = big.tile([P, MT], dt_f32, tag="rsum")
            nc.vector.reduce_sum(rsum[:], rs[:].rearrange("p (q m) -> p m q",
                                                          m=MT), axis=AX.X)
            den = big.tile([P, MT], dt_f32, tag="den")
            nc.vector.tensor_scalar_add(den[:], rsum[:],
                                        -float(np.exp(2.0)))
            ldb = big.tile([P, MT], dt_f32, tag="ldb")
            nc.scalar.activation(ldb[:], den[:], AFT.Ln)
            nc.gpsimd.dma_start(ld_d[:], ldb[:])

            # ---- positives: <Zn1, Zn2> ----
            pp = big.tile([P, NS // 2], dt_bf, tag="pp")
            nc.vector.tensor_mul(pp[:], zn_t[:, :NS // 2], zn_t[:, NS // 2:])
            posb = big.tile([P, 1], dt_f32, tag="posb")
            nc.vector.reduce_sum(posb[:], pp[:], axis=AX.X)
            nc.gpsimd.dma_start(pos_d[:], posb[:])

    nc.compile()
    return nc


def get_nc():
    if "nc" not in _CACHE:
        _CACHE["nc"] = _build()
    return _CACHE["nc"]


def make_in_maps(proj_1: np.ndarray, proj_2: np.ndarray):
    import ml_dtypes
    z = np.concatenate([np.asarray(proj_1), np.asarray(proj_2)], axis=0)
    zb = z.astype(ml_dtypes.bfloat16)
    # natural packed layout: [p, t, d] = z[128 t + p, d]
    znat = np.ascontiguousarray(zb.reshape(NT, P, P).transpose(1, 0, 2))
    ztr = np.ascontiguousarray(zb.T)
    in_maps = []
    for c in range(NCORES):
        myn = np.ascontiguousarray(
            zb[c * RB:(c + 1) * RB].reshape(MT, P, P).transpose(1, 0, 2))
        myt = np.ascontiguousarray(ztr[:, c * RB:(c + 1) * RB])
        in_maps.append({"znat": znat, "zt": ztr, "mynat": myn, "myt": myt})
    return in_maps


def finish(results) -> np.ndarray:
    ld_sum = 0.0
    pos_vals = []
    for r in results:
        ld_sum += float(np.asarray(r["out_ld"], dtype=np.float64).sum())
        pos_vals.append(float(np.asarray(r["out_pos"], dtype=np.float64).sum()))
    pos_dot = float(np.mean(pos_vals))
    loss = (ld_sum - 2.0 * INV_T * pos_dot) / float(NS)
    return np.float32(loss)


def kernel(proj_1: np.ndarray, proj_2: np.ndarray) -> np.ndarray:
    _ensure_paths()
    from concourse.bass_utils import run_bass_kernel_spmd
    nc = get_nc()
    in_maps = make_in_maps(proj_1, proj_2)
    res = run_bass_kernel_spmd(nc, in_maps, core_ids=list(range(NCORES)))
    return finish(res.results)


# revision 5
# speedup vs baseline: 1.1917x; 1.0833x over previous
"""NT-Xent / SimCLR contrastive loss on 8 Trainium2 NeuronCores.

Math (reference):
  z = concat(proj_1, proj_2)            # [2N, D], 2N=8192, D=128
  zn = z / ||z||                        # row L2-normalize
  sim = zn @ zn.T                       # [2N, 2N]
  denom_i   = sum_{j != i} exp(sim_ij / T)
  pos_i     = sim[i, (i+N) mod 2N]
  loss      = mean_i( log(denom_i) - pos_i / T )

Device decomposition (row-parallel over 8 cores, core c owns rows
[c*1024, (c+1)*1024)). Inputs per core: full z natural-packed
[128, 64, 128] (z[128t+p, d] at [p,t,d]) for row norms, full raw z^T
[128(d), 8192(sample)] for the GEMM moving operand, and the core's own
1024 columns of raw z^T for the stationary.

Normalization never materializes zn in natural layout: compact
1/||row|| ([128, nt] via square -> reduce -> ACT Sqrt -> DVE recip) is
re-ordered to sample order with one padded 128x128 DMA-transpose +
reshape DMA, bounced through DRAM, DMA-broadcast to all 128 partitions,
and applied with a single elementwise multiply against raw z^T. Four
independent 2048-column pipelines overlap DMA/DVE/ACT/transpose so the
GEMM starts ~10us in.

Gram phase: 128 matmuls (K=128, N=512, fp32 PSUM); exp + row-sum fused
on ScalarE via activation(Exp, scale=1/T, accum_out=...); exp values
are discarded. denom = rowsum - e^2 (self-sim), log on ScalarE.
positives: sum_i pos_i = 2*<Zn1,Zn2> elementwise, no diagonal
extraction. Host: loss = (sum log(denom) - (2/T)*<Zn1,Zn2>) / 8192.
"""

import numpy as np

P = 128          # partitions / feature dim
NS = 8192        # total samples (2N)
D = 128          # feature dim
NCORES = 8
RB = NS // NCORES    # 1024 rows per core
MT = RB // P         # 8 m-tiles per core
NT = NS // P         # 64 sample tiles
TEMP = 0.5
INV_T = 1.0 / TEMP   # 2.0
NFREE = 512          # matmul moving free dim (fp32 PSUM: one bank)
ACT_CHUNK = 2048     # ScalarE exp chunk (4 PSUM banks)
NCHUNK = NS // ACT_CHUNK   # 4 column chunks
CT = ACT_CHUNK // P        # 16 sample tiles per chunk

_CACHE = {}


def _ensure_paths():
    import sys
    for p in ("/root/.axon_site", "/root/.axon_site/_ro/trn_rl_repo",
              "/root/.axon_site/_ro/pypackages", "/opt/trn_rl_repo", "/opt/pypackages"):
        if p not in sys.path:
            sys.path.append(p)


def _build():
    _ensure_paths()
    import concourse.bass as bass
    import concourse.bacc as bacc
    import concourse.mybir as mybir
    import concourse.tile as tile

    dt_bf = mybir.dt.bfloat16
    dt_f32 = mybir.dt.float32
    AFT = mybir.ActivationFunctionType
    AX = mybir.AxisListType

    nc = bacc.Bacc("TRN2", target_bir_lowering=False, debug=False,
                   num_devices=NCORES)

    znat_d = nc.dram_tensor("znat", [P, NT, P], dt_bf, kind="ExternalInput")
    zt_d = nc.dram_tensor("zt", [P, NS], dt_bf, kind="ExternalInput")
    myt_d = nc.dram_tensor("myt", [P, RB], dt_bf, kind="ExternalInput")
    mynat_d = nc.dram_tensor("mynat", [P, MT, P], dt_bf, kind="ExternalInput")
    ld_d = nc.dram_tensor("out_ld", [P, MT], dt_f32, kind="ExternalOutput")
    pos_d = nc.dram_tensor("out_pos", [P, 1], dt_f32, kind="ExternalOutput")
    ninv_dram = nc.dram_tensor("ninv_row_scratch", [1, NS], dt_bf)
    minv_dram = nc.dram_tensor("minv_row_scratch", [1, RB], dt_bf)

    with tile.TileContext(nc) as tc:
        with (
            tc.tile_pool(name="big", bufs=1) as big,
            tc.tile_pool(name="work", bufs=2) as work,
            tc.tile_pool(name="psum", bufs=2, space=bass.MemorySpace.PSUM) as psum,
        ):
            zt = big.tile([P, NS], dt_bf, tag="zt")        # z^T raw
            zn_t = big.tile([P, NS], dt_bf, tag="zn_t")    # zn^T
            my_t = big.tile([P, RB], dt_bf, tag="my_t")    # own rows ^T
            rs = big.tile([P, MT * NCHUNK], dt_f32, tag="rs")

            # ---- own-rows norms + broadcast-normalize (early) ----
            myt_raw = big.tile([P, RB], dt_bf, tag="myt_raw")
            myna = big.tile([P, MT, P], dt_bf, tag="myna")
            zna = big.tile([P, NT, P], dt_bf, tag="zna")
            # bulk loads first, all on the HWDGE queue (cheap dispatch);
            # transposes follow on the same queue -> one xbar transition
            nc.sync.dma_start(myt_raw[:], myt_d[:])
            nc.sync.dma_start(myna[:], mynat_d[:])
            for h in range(NCHUNK):
                nc.sync.dma_start(zna[:, h * CT:(h + 1) * CT, :],
                                  znat_d[:, h * CT:(h + 1) * CT, :])
            for h in range(NCHUNK):
                nc.sync.dma_start(zt[:, h * ACT_CHUNK:(h + 1) * ACT_CHUNK],
                                  zt_d[:, h * ACT_CHUNK:(h + 1) * ACT_CHUNK])
            sqm = big.tile([P, MT, P], dt_bf, tag="sqm")
            nc.scalar.activation(sqm[:], myna[:], AFT.Square)
            ssm = big.tile([P, MT], dt_f32, tag="ssm")
            nc.vector.reduce_sum(ssm[:], sqm[:], axis=AX.X)
            snm = big.tile([P, MT], dt_f32, tag="snm")
            nc.scalar.activation(snm[:], ssm[:], AFT.Sqrt)
            minvb = big.tile([P, P], dt_bf, tag="minvb")   # cols 8: pad
            nc.gpsimd.memset(minvb[:, MT:], 1.0)
            ninvm = big.tile([P, MT], dt_f32, tag="ninvm")
            nc.vector.reciprocal(ninvm[:], snm[:])
            nc.vector.tensor_copy(minvb[:, :MT], ninvm[:])
            minv_t = big.tile([P, P], dt_bf, tag="minv_t")
            nc.sync.dma_start_transpose(minv_t[:], minvb[:])
            nc.gpsimd.dma_start(minv_dram[0:1, :], minv_t[0:MT, :])
            minv_b = big.tile([P, RB], dt_bf, tag="minv_b")
            nc.gpsimd.dma_start(minv_b[:],
                                minv_dram[0:1, :].broadcast_to([P, RB]))
            nc.vector.tensor_mul(my_t[:], myt_raw[:], minv_b[:])

            # ---- full-z norms + normalize, 4 independent chunks ----
            ss = big.tile([P, NT], dt_f32, tag="ss")
            sn = big.tile([P, NT], dt_f32, tag="sn")
            ninvf = big.tile([P, NT], dt_f32, tag="ninvf")
            ninv_b = big.tile([P, NS], dt_bf, tag="ninv_b")
            for h in range(NCHUNK):
                tsl = slice(h * CT, (h + 1) * CT)          # tile indices
                csl = slice(h * ACT_CHUNK, (h + 1) * ACT_CHUNK)  # columns
                sqg = work.tile([P, CT, P], dt_bf, tag="sq")
                nc.scalar.activation(sqg[:], zna[:, tsl, :], AFT.Square)
                nc.vector.reduce_sum(ss[:, tsl], sqg[:], axis=AX.X)
                nc.scalar.activation(sn[:, tsl], ss[:, tsl], AFT.Sqrt)
                nc.vector.reciprocal(ninvf[:, tsl], sn[:, tsl])
                nvb = work.tile([P, P], dt_bf, tag="nvb")  # cols CT: pad
                nc.gpsimd.memset(nvb[:, CT:], 1.0)
                nc.vector.tensor_copy(nvb[:, :CT], ninvf[:, tsl])
                nvt = work.tile([P, P], dt_bf, tag="nvt")
                nc.sync.dma_start_transpose(nvt[:], nvb[:])
                nc.gpsimd.dma_start(ninv_dram[0:1, csl], nvt[0:CT, :])
                nc.gpsimd.dma_start(ninv_b[:, csl],
                                    ninv_dram[0:1, csl].broadcast_to(
                                        [P, ACT_CHUNK]))
                nc.vector.tensor_mul(zn_t[:, csl], zt[:, csl],
                                     ninv_b[:, csl])

            # ---- gram rows + fused exp/row-sum ----
            nmm = ACT_CHUNK // NFREE               # 4 matmuls per chunk
            for q in range(NCHUNK):
                for m in range(MT):
                    ps = psum.tile([P, ACT_CHUNK], dt_f32, tag="ps")
                    for j in range(nmm):
                        c0 = q * ACT_CHUNK + j * NFREE
                        nc.tensor.matmul(ps[:, j * NFREE:(j + 1) * NFREE],
                                         my_t[:, m * P:(m + 1) * P],
                                         zn_t[:, c0:c0 + NFREE],
                                         start=True, stop=True)
                    esc = work.tile([P, ACT_CHUNK], dt_bf, tag="esc")
                    idx = q * MT + m
                    nc.scalar.activation(esc[:], ps[:], AFT.Exp, scale=INV_T,
                                         accum_out=rs[:, idx:idx + 1])

            # ---- denominators -> log ----
            rsum = big.tile([P, MT], dt_f32, tag="rsum")
            nc.vector.reduce_sum(rsum[:], rs[:].rearrange("p (q m) -> p m q",
                                                          m=MT), axis=AX.X)
            den = big.tile([P, MT], dt_f32, tag="den")
            nc.vector.tensor_scalar_add(den[:], rsum[:],
                                        -float(np.exp(2.0)))
            ldb = big.tile([P, MT], dt_f32, tag="ldb")
            nc.scalar.activation(ldb[:], den[:], AFT.Ln)
            nc.gpsimd.dma_start(ld_d[:], ldb[:])

            # ---- positives: <Zn1, Zn2> ----
            pp = big.tile([P, NS // 2], dt_bf, tag="pp")
            nc.vector.tensor_mul(pp[:], zn_t[:, :NS // 2], zn_t[:, NS // 2:])
            posb = big.tile([P, 1], dt_f32, tag="posb")
            nc.vector.reduce_sum(posb[:], pp[:], axis=AX.X)
            nc.gpsimd.dma_start(pos_d[:], posb[:])

    nc.compile()
    return nc


def get_nc():
    if "nc" not in _CACHE:
        _CACHE["nc"] = _build()
    return _CACHE["nc"]


def make_in_maps(proj_1: np.ndarray, proj_2: np.ndarray):
    import ml_dtypes
    z = np.concatenate([np.asarray(proj_1), np.asarray(proj_2)], axis=0)
    zb = z.astype(ml_dtypes.bfloat16)
    # natural packed layout: [p, t, d] = z[128 t + p, d]
    znat = np.ascontiguousarray(zb.reshape(NT, P, P).transpose(1, 0, 2))
    ztr = np.ascontiguousarray(zb.T)
    in_maps = []
    for c in range(NCORES):
        myn = np.ascontiguousarray(
            zb[c * RB:(c + 1) * RB].reshape(MT, P, P).transpose(1, 0, 2))
        myt = np.ascontiguousarray(ztr[:, c * RB:(c + 1) * RB])
        in_maps.append({"znat": znat, "zt": ztr, "mynat": myn, "myt": myt})
    return in_maps


def finish(results) -> np.ndarray:
    ld_sum = 0.0
    pos_vals = []
    for r in results:
        ld_sum += float(np.asarray(r["out_ld"], dtype=np.float64).sum())
        pos_vals.append(float(np.asarray(r["out_pos"], dtype=np.float64).sum()))
    pos_dot = float(np.mean(pos_vals))
    loss = (ld_sum - 2.0 * INV_T * pos_dot) / float(NS)
    return np.float32(loss)


def kernel(proj_1: np.ndarray, proj_2: np.ndarray) -> np.ndarray:
    _ensure_paths()
    from concourse.bass_utils import run_bass_kernel_spmd
    nc = get_nc()
    in_maps = make_in_maps(proj_1, proj_2)
    res = run_bass_kernel_spmd(nc, in_maps, core_ids=list(range(NCORES)))
    return finish(res.results)
